# revision 1
# baseline (speedup 1.0000x reference)
"""AdaptiveAttentionLSTMCell fused kernel for one TRN2 chip (8 NeuronCores).

Math note: the reference applies softmax over a size-1 axis (zt is [B, K+1, 1],
softmax(axis=-1)), which is identically 1.0 for finite inputs. Hence
ct = sum_k v_expand[:, k, :] = v_seq.sum(axis=1) + st exactly, and the
W_z / U_z / W_h attention projections never affect the output. The kernel
therefore computes:

    z  = h_tm @ W_gates + inputs @ U_gates + b_gates          [B, 5U]
    ft,it,ot,gt = sigmoid(f,i,o,g);  at = tanh(a)
    mt = m_tm * ft + it * at
    tm = tanh(mt); ht = ot * tm; st = gt * tm
    out = (ht + st + v_seq.sum(1), ht, mt)     # (ot+gt)*tm == ht+st

Distribution: 2-way data-parallel over batch x 4-way parallel over the unit
dim (each core owns all 5 gate blocks for its 256 units; no collective).

Precision plan (validated numerically vs the f32 reference, worst rel_l2
~1.6e-2 < 2e-2 gate):
  - f, i, g gate matmuls run in fp8 e4m3 with DoubleRow perf mode (two
    k-planes per pass -> ~1.8x PE throughput). Host scales A by 32 and W by
    512; the 1/16384 dequant is folded into the ACT sigmoid scale.
  - a (tanh) and o gates are error-critical (tanh'~1 and ht's small norm)
    and stay bf16.
  - v_seq is int8 (host scale sv), halving its DMA; the 49-deep reduction
    runs on DVE as an exact int16 add tree, dequantized by an ACT copy.

Per-core schedule: phase A (fp8 DoubleRow, f|i|g = 768 cols) for batch
group 0 then group 1, then phase B (bf16, a|o = 512 cols) per group. Input
DMA is split across both HWDGE rings: weights/activations on the sync(SP)
ring, v/m/outputs on the scalar(ACT) ring. Gate epilogues run group-wide on
[128, 4, 256] tiles; outputs are bf16 (host upcasts).
"""

import numpy as np
import ml_dtypes

# Problem shape (hardcoded per the harness contract).
B, D_IN, UNITS, KF = 2048, 2048, 1024, 49
N_CORES = 8
PB, PU = 2, 4                 # batch shards x unit shards
B_L = B // PB                 # 1024 batch rows per core
U_L = UNITS // PU             # 256 units per core
K = UNITS + D_IN              # 3072 contraction dim
P = 128                       # partitions
NB_T = B_L // P               # 8 batch tiles
NK_T = K // P                 # 24 k tiles
KS = NK_T // 2                # 12 DoubleRow k supertiles
GRP = 4                       # batch tiles per phase group
N8 = 3 * U_L                  # 768 fp8 gate cols per core (f, i, g)
NB = 2 * U_L                  # 512 bf16 gate cols per core (a, o)
SA, SW = 32.0, 512.0          # host quant scales for acts / weights
SDQ = 1.0 / (SA * SW)         # dequant folded into ACT scale
BF16 = ml_dtypes.bfloat16
E4M3 = ml_dtypes.float8_e4m3

_NC_CACHE = {}


def _build_nc(with_bias, sv):
    import concourse.bacc as bacc
    import concourse.mybir as mybir
    import concourse.tile as tile

    dt = mybir.dt
    f32, bf, f8, i8, i16 = dt.float32, dt.bfloat16, dt.float8e4, dt.int8, dt.int16
    Sig = mybir.ActivationFunctionType.Sigmoid
    Tanh = mybir.ActivationFunctionType.Tanh
    DR = mybir.MatmulPerfMode.DoubleRow
    nc = bacc.Bacc("TRN2", target_bir_lowering=False, debug=False)

    a8 = nc.dram_tensor("a8", [P, NK_T * B_L], f8, kind="ExternalInput").ap()
    ab = nc.dram_tensor("ab", [P, NK_T * B_L], bf, kind="ExternalInput").ap()
    w8 = nc.dram_tensor("w8", [P, NK_T * N8], f8, kind="ExternalInput").ap()
    wb = nc.dram_tensor("wb", [P, NK_T * NB], bf, kind="ExternalInput").ap()
    m = nc.dram_tensor("m", [B_L, U_L], bf, kind="ExternalInput").ap()
    v = nc.dram_tensor("v", [B_L, KF, U_L], i8, kind="ExternalInput").ap()
    if with_bias:
        bb = nc.dram_tensor("bb", [P, 5 * U_L], f32, kind="ExternalInput").ap()
    o0 = nc.dram_tensor("o0", [B_L, U_L], bf, kind="ExternalOutput").ap()
    o1 = nc.dram_tensor("o1", [B_L, U_L], bf, kind="ExternalOutput").ap()
    o2 = nc.dram_tensor("o2", [B_L, U_L], bf, kind="ExternalOutput").ap()

    GROUPS = (tuple(range(0, GRP)), tuple(range(GRP, NB_T)))

    def grp_dram(ap_, g):
        # [512, 256] dram rows of group g viewed as [128, 4, 256]
        return ap_[g * GRP * P:(g + 1) * GRP * P, :].rearrange(
            "(i p) u -> p i u", p=P)

    with tile.TileContext(nc) as tc:
        with (
            tc.tile_pool(name="resident", bufs=1) as rp,
            tc.tile_pool(name="vload", bufs=3) as vp,
            tc.tile_pool(name="vsum16", bufs=2) as sp,
            tc.tile_pool(name="grp", bufs=2) as gp,
            tc.tile_pool(name="psum", bufs=8, space="PSUM") as pp,
        ):
            # resident inputs; DoubleRow operands are [P, super, plane, n]
            a8_sb = rp.tile([P, KS, 2, B_L], f8)
            w8_sb = rp.tile([P, KS, 2, N8], f8)
            ab_sb = rp.tile([P, NK_T, B_L], bf)
            wb_sb = rp.tile([P, NK_T, NB], bf)

            # graded chunks: small first chunk gets the PE started early;
            # phase-B (bf16) tensors stream in behind the fp8 ones.
            CH8 = ((0, 1), (1, 2), (2, 4), (4, 8), (8, 12))
            if with_bias:
                bb_sb = rp.tile([P, 5 * U_L], f32)
                nc.sync.dma_start(bb_sb[:], bb[:])

            # scalar ring: v tiles lead (DVE/Pool trees consume them from
            # t~10us); m is only needed by the group epilogues much later.
            vts = {}
            m_gs = {}

            def v_dma(bt):
                bs = slice(bt * P, (bt + 1) * P)
                vt = vp.tile([P, KF, U_L], i8, tag="v", name=f"v{bt}")
                nc.scalar.dma_start(vt[:], v[bs, :, :])
                vts[bt] = vt

            def m_dma_sync(g):
                m_g = gp.tile([P, GRP, U_L], bf, tag="m", name=f"m_g{g}")
                nc.sync.dma_start(m_g[:], grp_dram(m, g))
                m_gs[g] = m_g

            # sync ring: first fp8 chunk -> m (a late m DMA head-blocks the
            # DVE stream at the hoisted m*ft ops) -> rest of the fp8 weights
            # -> phase-B weights, finely interleaved so the PE k-loops are
            # never starved.
            s0, s1 = CH8[0]
            nc.sync.dma_start(w8_sb[:, s0:s1, :, :], w8[:, s0*2*N8:s1*2*N8])
            nc.sync.dma_start(a8_sb[:, s0:s1, :, :], a8[:, s0*2*B_L:s1*2*B_L])
            m_dma_sync(0)
            m_dma_sync(1)
            for s0, s1 in CH8[1:]:
                nc.sync.dma_start(w8_sb[:, s0:s1, :, :],
                                  w8[:, s0 * 2 * N8:s1 * 2 * N8])
                nc.sync.dma_start(a8_sb[:, s0:s1, :, :],
                                  a8[:, s0 * 2 * B_L:s1 * 2 * B_L])
            for k0, k1 in ((0, 8), (8, 16), (16, 24)):
                nc.sync.dma_start(wb_sb[:, k0:k1, :],
                                  wb[:, k0 * NB:k1 * NB])
                nc.sync.dma_start(ab_sb[:, k0:k1, :],
                                  ab[:, k0 * B_L:k1 * B_L])
            # scalar ring: v0..v4 upfront (waits resolve before the phase-A
            # activations are ready); v5..v7 are emitted later, between the
            # phase-A and phase-B activations.
            for bt in (0, 1, 2, 3, 4):
                v_dma(bt)

            # ---- gate tiles per group ----
            G = {}
            for g in range(2):
                G[g] = dict(
                    fi=gp.tile([P, GRP, 2 * U_L], bf, tag="fi", name=f"fi{g}"),
                    gt=gp.tile([P, GRP, U_L], bf, tag="gt", name=f"gt{g}"),
                    at=gp.tile([P, GRP, U_L], bf, tag="at", name=f"at{g}"),
                    ot=gp.tile([P, GRP, U_L], bf, tag="ot", name=f"ot{g}"),
                    vs=gp.tile([P, GRP, U_L], bf, tag="vs", name=f"vs{g}"),
                    tm=gp.tile([P, GRP, U_L], bf, tag="tm", name=f"tm{g}"),
                )

            def phaseA(g):
                """fp8 DoubleRow matmuls for f|i (512 cols) and g (256)."""
                group = GROUPS[g]
                zfi, zg = {}, {}
                for bt in group:
                    zfi[bt] = pp.tile([P, 512], f32, tag="z", name=f"zfi{bt}")
                    zg[bt] = pp.tile([P, 512], f32, tag="z", name=f"zg{bt}")
                for s in range(KS):
                    st_, sp_ = (s == 0), (s == KS - 1)
                    for bt in group:
                        lhsT = a8_sb[:, s, :, bt * P:(bt + 1) * P]
                        nc.tensor.matmul(
                            zfi[bt][:], lhsT, w8_sb[:, s, :, 0:512],
                            start=st_, stop=sp_, perf_mode=DR)
                        nc.tensor.matmul(
                            zg[bt][:, 0:256], lhsT, w8_sb[:, s, :, 512:768],
                            start=st_, stop=sp_, perf_mode=DR)
                if with_bias:
                    for bt in group:
                        nc.vector.tensor_add(zfi[bt][:], zfi[bt][:],
                                             bfi_sb[:])
                        nc.vector.tensor_add(zg[bt][:, 0:256], zg[bt][:, 0:256],
                                             bg_sb[:, 0:256])
                gg = G[g]
                for i, bt in enumerate(group):
                    nc.scalar.activation(gg["fi"][:, i, :], zfi[bt][:], Sig,
                                         scale=SDQ)
                    nc.scalar.activation(gg["gt"][:, i, :], zg[bt][:, 0:256],
                                         Sig, scale=SDQ)

            def phaseB_mm(g):
                """bf16 matmuls for a|o (512 cols)."""
                group = GROUPS[g]
                zao = {}
                for bt in group:
                    zao[bt] = pp.tile([P, 512], f32, tag="z", name=f"zao{bt}")
                for k in range(NK_T):
                    st_, sp_ = (k == 0), (k == NK_T - 1)
                    for bt in group:
                        nc.tensor.matmul(
                            zao[bt][:],
                            ab_sb[:, k, bt * P:(bt + 1) * P],
                            wb_sb[:, k, :],
                            start=st_, stop=sp_)
                if with_bias:
                    for bt in group:
                        nc.vector.tensor_add(zao[bt][:], zao[bt][:], bao_sb[:])
                return zao

            def phaseB_act(g, zao, i, bt):
                gg = G[g]
                nc.scalar.activation(gg["at"][:, i, :], zao[bt][:, 0:256], Tanh)
                nc.scalar.activation(gg["ot"][:, i, :], zao[bt][:, 256:512], Sig)

            A = nc.vector.tensor_add
            M = nc.vector.tensor_mul

            s16s = {}

            def tree(bt):
                """49-deep int8 v reduction, all on DVE (GPSIMD shares DVE's
                2nd SBUF port via an exclusive lock, so Pool tensor ops would
                just steal DVE time). int8 L0 runs at 1x, bf16 uppers at 2x.
                The raw sum lands in the group tile; sv dequant is fused into
                the epilogue scalar_tensor_tensor."""
                g, i = divmod(bt, GRP)
                vt = vts[bt]
                s = sp.tile([P, 24, U_L], bf, tag="s16", name=f"s16_{bt}")
                A(s[:, 0:24, :], vt[:, 0:24, :], vt[:, 24:48, :])
                A(s[:, 23, :], s[:, 23, :], vt[:, 48, :])   # odd 49th slice
                A(s[:, 0:12, :], s[:, 0:12, :], s[:, 12:24, :])
                A(s[:, 0:6, :], s[:, 0:6, :], s[:, 6:12, :])
                A(s[:, 0:3, :], s[:, 0:3, :], s[:, 3:6, :])
                A(s[:, 0, :], s[:, 0, :], s[:, 1, :])
                A(G[g]["vs"][:, i, :], s[:, 0, :], s[:, 2, :])

            def chains1(g):
                gg = G[g]
                m_g = m_gs[g]
                fi, at, tm = gg["fi"], gg["at"], gg["tm"]
                ft = fi[:, :, 0:U_L]
                it = fi[:, :, U_L:2 * U_L]
                M(m_g[:], m_g[:], ft)                        # m*ft
                M(it, it, at[:])                             # it*at
                A(m_g[:], m_g[:], it)                        # mt (in m_g)
                nc.scalar.dma_start(grp_dram(o2, g), m_g[:])
                nc.scalar.activation(tm[:], m_g[:], Tanh)

            add_, mul_ = mybir.AluOpType.add, mybir.AluOpType.mult

            def chains2_final(g):
                gg = G[g]
                fi, ot, gt, tm, vs = (gg[k] for k in
                                      ("fi", "ot", "gt", "tm", "vs"))
                ft = fi[:, :, 0:U_L]
                it = fi[:, :, U_L:2 * U_L]
                M(it, ot[:], tm[:])                          # ht (in it)
                nc.scalar.dma_start(grp_dram(o1, g), it)
                A(ot[:], ot[:], gt[:])                       # ot+gt
                M(ft, ot[:], tm[:])                          # ht+st (in ft)
                nc.vector.scalar_tensor_tensor(              # + sv*vsum_raw
                    ft, vs[:], sv, ft, op0=mul_, op1=add_)
                nc.scalar.dma_start(grp_dram(o0, g), ft)

            if with_bias:
                # pre-scaled psum-resident bias rows (rarely used: b=0)
                bfi_sb = rp.tile([P, 512], f32)
                bg_sb = rp.tile([P, 512], f32)
                bao_sb = rp.tile([P, 512], f32)
                nc.vector.tensor_scalar_mul(bfi_sb[:], bb_sb[:, 0:512],
                                            SA * SW)
                nc.vector.tensor_scalar_mul(bg_sb[:], bb_sb[:, 512:1024],
                                            SA * SW)
                nc.vector.tensor_copy(bao_sb[:], bb_sb[:, 768:1280])

            # ---- schedule; emission order = per-engine FIFO order ----
            phaseA(0)
            phaseA(1)
            # v5..v7 dispatches: buffer-free waits (trees 2..4) all resolve
            # before the phase-B(0) activations behind them become ready.
            for bt in (5, 6, 7):
                v_dma(bt)
            tree(0)
            tree(1)
            tree(2)
            tree(3)
            zao0 = phaseB_mm(0)
            for i, bt in enumerate(GROUPS[0]):
                phaseB_act(0, zao0, i, bt)
            tree(4)
            chains1(0)
            chains2_final(0)
            zao1 = phaseB_mm(1)
            for i, bt in enumerate(GROUPS[1]):
                phaseB_act(1, zao1, i, bt)
            tree(5)
            tree(6)
            tree(7)
            chains1(1)
            chains2_final(1)

    nc.compile()
    return nc


def _get_nc(with_bias, sv):
    key = (bool(with_bias), float(sv))
    if key not in _NC_CACHE:
        _NC_CACHE[key] = _build_nc(bool(with_bias), float(sv))
    return _NC_CACHE[key]


def _prepare_in_maps(inputs):
    x = np.asarray(inputs["inputs"], np.float32)
    h = np.asarray(inputs["h_tm"], np.float32)
    m = np.asarray(inputs["m_tm"], np.float32)
    v = np.asarray(inputs["v_seq"], np.float32)
    Wg = np.asarray(inputs["W_gates"], np.float32)
    Ug = np.asarray(inputs["U_gates"], np.float32)
    bg = np.asarray(inputs["b_gates"], np.float32)

    with_bias = bool(np.any(bg))
    A_T = np.concatenate([h, x], axis=1).T.astype(np.float32)  # [K, B]
    W_full = np.concatenate([Wg, Ug], axis=0)                  # [K, 5U]
    sv = float(np.abs(v).max()) / 127.0
    v_i8 = np.clip(np.round(v / sv), -127, 127).astype(np.int8)

    in_maps = []
    for c in range(N_CORES):
        pb, pu = divmod(c, PU)
        bsl = slice(pb * B_L, (pb + 1) * B_L)
        u = np.arange(pu * U_L, (pu + 1) * U_L)
        # fp8 block [f, i, g]; bf16 block [a, o]
        # (reference stacks gates as [f, i, o, g, a])
        cols8 = np.concatenate([j * UNITS + u for j in (0, 1, 3)])
        colsb = np.concatenate([j * UNITS + u for j in (4, 2)])
        aTl = A_T[:, bsl]                                      # [3072, 1024]
        # [P, KS, 2, B_L]: k = s*256 + plane*128 + p
        a8l = (aTl * SA).astype(E4M3).reshape(KS, 2, P, B_L).transpose(2, 0, 1, 3)
        abl = aTl.astype(BF16).reshape(NK_T, P, B_L).transpose(1, 0, 2)
        w8l = (W_full[:, cols8] * SW).astype(E4M3).reshape(
            KS, 2, P, N8).transpose(2, 0, 1, 3)
        wbl = W_full[:, colsb].astype(BF16).reshape(
            NK_T, P, NB).transpose(1, 0, 2)
        im = {
            "a8": np.ascontiguousarray(a8l.reshape(P, NK_T * B_L)),
            "ab": np.ascontiguousarray(abl.reshape(P, NK_T * B_L)),
            "w8": np.ascontiguousarray(w8l.reshape(P, NK_T * N8)),
            "wb": np.ascontiguousarray(wbl.reshape(P, NK_T * NB)),
            "m": np.ascontiguousarray(m[bsl, pu * U_L:(pu + 1) * U_L].astype(BF16)),
            "v": np.ascontiguousarray(v_i8[bsl, :, pu * U_L:(pu + 1) * U_L]),
        }
        if with_bias:
            cols = np.concatenate([cols8, colsb])
            im["bb"] = np.ascontiguousarray(
                np.broadcast_to(bg[cols], (P, 5 * U_L)).astype(np.float32))
        in_maps.append(im)
    return in_maps, with_bias, sv


def _assemble(results):
    outs = []
    for name in ("o0", "o1", "o2"):
        full = np.empty((B, UNITS), np.float32)
        for c in range(N_CORES):
            pb, pu = divmod(c, PU)
            full[pb * B_L:(pb + 1) * B_L, pu * U_L:(pu + 1) * U_L] = \
                np.asarray(results[c][name]).astype(np.float32)
        outs.append(full)
    return tuple(outs)


def _run(inputs, **spmd_kwargs):
    from concourse.bass_utils import run_bass_kernel_spmd

    in_maps, with_bias, sv = _prepare_in_maps(inputs)
    nc = _get_nc(with_bias, sv)
    res = run_bass_kernel_spmd(nc, in_maps, core_ids=list(range(N_CORES)),
                               **spmd_kwargs)
    return _assemble(res.results), res


def kernel(**inputs):
    outs, _ = _run(inputs)
    return outs

